# revision 1
# baseline (speedup 1.0000x reference)
"""ChannelTimeAttention Trainium2 kernel.

Reference computation (per (b, c) pair, all independent):
    pooled = AdaptiveAvgPool(x[b, :, c]) -> [t, 8*8]      (7x7 block means)
    q = pooled @ Wq + bq ; k = pooled @ Wk + bk           [t, 32]
    att = softmax(q @ k.T / sqrt(t))                      [t, t]
    out[b, :, c] = att @ x[b, :, c].reshape(t, h*w)

Sharding: data-parallel over b — one batch element per NeuronCore (8 cores).
Each core streams its x slice [t=16, c=64, h=56, w=56] through SBUF once in
8 "packs" of 8 channels, with partition layout (c_local*16 + t).  Per pack:
  DVE two-stage strided reduce  -> pooled sums [128, 64]
  PE  transpose + 2 matmuls     -> q^T, k^T [32, 128]
  PE  full 128x128 cross-score matmul + additive block-diag mask + softmax
  PE  transpose(att) -> block-diagonal lhsT, then att @ v in 7 N=448 chunks
  DMA out.
1/49 (pool mean), 1/sqrt(16) (score scale) are folded into Wq/bq/Wk on host.
"""

import numpy as np

B, T, C, H, W = 8, 16, 64, 56, 56
DS = 8
DIN = DS * DS  # 64
DOUT = 32
HW = H * W  # 3136
CG = 8  # channels per pack
NPACK = C // CG  # 8
P = CG * T  # 128 partitions
NCH = 7  # output free-dim chunks per pack
CHN = HW // NCH  # 448
N_CORES = 8
MASK_NEG = -30.0

# att @ v matmul dtype: float32r streams 1 row/cycle (vs 4 for float32)
USE_F32R = True


def _build_nc():
    import concourse.bacc as bacc
    import concourse.tile as tile
    from concourse import mybir
    from contextlib import ExitStack

    f32 = mybir.dt.float32
    # Bacc (not raw Bass): its compile() runs generate_event_semaphores /
    # move_matmul_waits_to_ldweights, which legalize multi-wait instructions
    # down to the 1-sync-wait-per-instruction TRN2 codegen limit.
    nc = bacc.Bacc(trn_type="TRN2", num_swdge_queues=2)

    x_h = nc.dram_tensor("x", [T, C, H, W], f32, kind="ExternalInput")
    # all small constants packed into ONE [128, 194] array (one DMA with
    # >=512B per-partition rows — six separate tiny DMAs cost ~25us of
    # latency-bound sub-512B descriptors):
    #   cols 0:128   mask, 128:160 wq (rows 0:64), 160:192 wk (rows 0:64),
    #   col 192 bq (rows 0:32), col 193 bk (rows 0:32)
    cn_h = nc.dram_tensor("consts", [P, 194], f32, kind="ExternalInput")
    out_h = nc.dram_tensor("out", [T, C, H, W], f32, kind="ExternalOutput")

    X = mybir.AxisListType.X
    Exp = mybir.ActivationFunctionType.Exp

    with ExitStack() as ctx:
        tc = ctx.enter_context(tile.TileContext(nc))
        singles = ctx.enter_context(tc.tile_pool(name="singles", bufs=1))
        # bufs=NPACK: every v-DMA writes a fresh slot, so no WAW wait back to
        # a previous pack's multi-queue DMA (DMA descriptors allow very few
        # sync-waits, and Tile doesn't elide transitively-covered DMA sems).
        vpool = ctx.enter_context(tc.tile_pool(name="vpool", bufs=NPACK))
        vrpool = ctx.enter_context(tc.tile_pool(name="vrpool", bufs=3))
        opool = ctx.enter_context(tc.tile_pool(name="opool", bufs=3))
        small = ctx.enter_context(tc.tile_pool(name="small", bufs=2))
        attpool = ctx.enter_context(tc.tile_pool(name="attpool", bufs=3))
        psA = ctx.enter_context(tc.tile_pool(name="psA", bufs=1, space="PSUM"))
        psB = ctx.enter_context(tc.tile_pool(name="psB", bufs=3, space="PSUM"))

        # PE-consumed constants are re-materialized through a DVE copy: the
        # gen3 LDWEIGHTS struct carries at most ONE sync-wait, so matmuls
        # must never need both a DMA-sem and a compute-sem wait.  Routing the
        # consts through DVE makes every PE wait a (cumulative) compute sem.
        consts = singles.tile([P, 194], f32)
        nc.scalar.dma_start(out=consts, in_=cn_h[:])
        mask = consts[:, 0:128]
        wq = consts[0:DIN, 128:160]
        wk = consts[0:DIN, 160:192]
        bq = consts[0:DOUT, 192:193]
        bk = consts[0:DOUT, 193:194]
        ident = singles.tile([P, P], f32)

        x_ap = x_h[:]
        out_ap = out_h[:]

        # All 8 input DMAs issued up-front into dedicated v slots, alternating
        # the sync-HWDGE ring and the gpsimd-SWDGE queues.  One queue/ring
        # sustains only ~90-140 GB/s of this pattern; the pipes run in
        # parallel (HBM cap ~358 GB/s/core).
        mm_dt = mybir.dt.float32r if USE_F32R else f32
        v_tiles = []
        for p in range(NPACK):
            c0 = p * CG
            # v[(t*8 + c_l), h*w] = x[t, c0+c_l, h, w]  — t-MAJOR partition
            # order, so the DMA walks DRAM nearly sequentially (100KB runs);
            # the c-major order only reached ~165 GB/s vs ~266 GB/s for this.
            # (Keep each DMA full-128-partition — 64-partition halves run at
            # half port bandwidth.)
            v = vpool.tile([P, HW], f32, tag="v")
            src = x_ap[:, c0 : c0 + CG, :, :].rearrange("t c h w -> t c (h w)")
            # (column-splitting pack 0 / pack 7 across both pipes was tried
            # and REGRESSED: half-width descriptors are less efficient)
            # Inputs ride THREE pipes: sync + gpsimd + the scalar HWDGE ring
            # (idle after the consts DMA; input loads carry no sem-waits so
            # they cannot head-of-line block ACT compute dispatches).
            eng = [nc.sync, nc.gpsimd, nc.scalar][p % 3]
            eng.dma_start(out=v[:], in_=src)
            v_tiles.append(v)

        # identity built on-chip (gpsimd memset + affine_select) — no DMA,
        # ready within a few us of kernel start
        from concourse.masks import make_identity

        make_identity(nc, ident[:])

        # Two-stage software pipeline: stage 1 (pool -> q/k -> scores ->
        # softmax -> att^T, plus the fp32r rounding of v) for pack p is
        # emitted BEFORE stage 2 (att @ v -> out DMA) of pack p-1, so the
        # next pack's DVE/ACT work is prioritized ahead of the previous
        # pack's PSUM evacuation and the per-pack cross-engine dependency
        # cycle spans two packs instead of one.
        stage2 = []  # (pack_idx, v_mm, attT)

        def emit_stage1(p):
            v = v_tiles[p]
            # round v to fp32r for the PE (ACT)
            v_mm = vrpool.tile([P, HW], mm_dt, tag="vr")
            nc.scalar.copy(out=v_mm, in_=v)

            # ---- adaptive avg pool (sum; /49 folded into weights) ----
            tmp = small.tile([P, H, DS], f32, tag="tmp")
            nc.vector.reduce_sum(
                out=tmp[:],
                in_=v[:].rearrange("p (h j vv) -> p h j vv", h=H, j=DS, vv=7),
                axis=X,
            )
            pooled = small.tile([P, DS, DS], f32, tag="pooled")
            nc.vector.reduce_sum(
                out=pooled[:],
                in_=tmp[:].rearrange("p (i u) j -> p i j u", i=DS, u=7),
                axis=X,
            )

            # ---- pooled^T via PE so q/k matmuls contract over d_in ----
            pooledT_ps = psA.tile([DIN, P], f32, tag="pooledT_ps")
            nc.tensor.transpose(
                pooledT_ps, pooled[:].rearrange("p i j -> p (i j)"), ident
            )
            pooledT = small.tile([DIN, P], f32, tag="pooledT")
            nc.scalar.copy(pooledT, pooledT_ps)

            # ---- q^T, k^T [32, 128] ----
            qT_ps = psA.tile([DOUT, P], f32, tag="qT_ps")
            nc.tensor.matmul(qT_ps, lhsT=wq, rhs=pooledT, start=True, stop=True)
            kT_ps = psA.tile([DOUT, P], f32, tag="kT_ps")
            nc.tensor.matmul(kT_ps, lhsT=wk, rhs=pooledT, start=True, stop=True)
            qT = small.tile([DOUT, P], f32, tag="qT")
            nc.vector.tensor_scalar_add(out=qT, in0=qT_ps, scalar1=bq)
            kT = small.tile([DOUT, P], f32, tag="kT")
            nc.vector.tensor_scalar_add(out=kT, in0=kT_ps, scalar1=bk)

            # ---- full cross scores [128, 128]; only diag blocks survive mask
            sc_ps = psA.tile([P, P], f32, tag="sc_ps")
            nc.tensor.matmul(sc_ps, lhsT=qT, rhs=kT, start=True, stop=True)
            scm = small.tile([P, P], f32, tag="scm")
            nc.vector.tensor_add(out=scm, in0=sc_ps, in1=mask)

            # ---- softmax along free dim ----
            negm = small.tile([P, 1], f32, tag="negm")
            nc.vector.reduce_max(out=negm, in_=scm, axis=X, negate=True)
            e = small.tile([P, P], f32, tag="e")
            ssum = small.tile([P, 1], f32, tag="ssum")
            nc.scalar.activation(
                out=e, in_=scm, func=Exp, bias=negm, scale=1.0, accum_out=ssum
            )
            rinv = small.tile([P, 1], f32, tag="rinv")
            nc.vector.reciprocal(rinv, ssum)
            att = small.tile([P, P], f32, tag="att")
            nc.vector.tensor_scalar_mul(out=att, in0=e, scalar1=rinv)

            # ---- att^T (block-diagonal) becomes the stationary operand ----
            attT_ps = psA.tile([P, P], f32, tag="attT_ps")
            nc.tensor.transpose(attT_ps, att, ident)
            attT = attpool.tile([P, P], mm_dt, tag="attT")
            nc.scalar.copy(attT, attT_ps)
            stage2.append((p, v_mm, attT))

        def emit_stage2(p, v_mm, attT):
            c0 = p * CG
            o = opool.tile([P, HW], f32, tag="o")
            # claim the o slot with a cheap DVE op: it absorbs the WAR wait
            # on the out-DMA that previously read this slot
            nc.vector.memset(o[:, 0:1], 0.0)
            for ch in range(NCH):
                sl = slice(ch * CHN, (ch + 1) * CHN)
                ops = psB.tile([P, CHN], f32, tag="ochunk")
                nc.tensor.matmul(
                    ops, lhsT=attT[:], rhs=v_mm[:, sl], start=True, stop=True
                )
                # split PSUM->SBUF evacuation between DVE and ACT
                if ch % 2 == 0:
                    nc.vector.tensor_copy(out=o[:, sl], in_=ops)
                else:
                    nc.scalar.copy(out=o[:, sl], in_=ops)

            # outs alternate the two DMA pipes; t-major order writes DRAM
            # nearly sequentially as well
            dst = out_ap[:, c0 : c0 + CG, :, :].rearrange("t c h w -> t c (h w)")
            eng = nc.gpsimd if p % 2 == 0 else nc.sync
            eng.dma_start(out=dst, in_=o[:])

        for p in range(NPACK):
            emit_stage1(p)
            if p >= 1:
                emit_stage2(*stage2[p - 1])
        emit_stage2(*stage2[NPACK - 1])

    nc.compile()
    return nc


def _host_consts(Wq, bq, Wk, bk):
    # fold pool-mean 1/49 into both weight mats; fold score 1/sqrt(t)=1/4
    # into the q side (weights AND bias)
    wq_eff = (Wq / (49.0 * 4.0)).astype(np.float32)
    bq_eff = (bq / 4.0).astype(np.float32)
    wk_eff = (Wk / 49.0).astype(np.float32)
    bk_eff = bk.astype(np.float32)
    # t-major partition order: row i = (t=i//8, c=i%8); attention pairs
    # (i, j) belong to the same channel iff i%8 == j%8
    idx = np.arange(P)
    same_c = np.equal.outer(idx % CG, idx % CG)
    mask = np.where(same_c, 0.0, MASK_NEG).astype(np.float32)
    consts = np.zeros((P, 194), dtype=np.float32)
    consts[:, 0:128] = mask
    consts[0:DIN, 128:160] = wq_eff
    consts[0:DIN, 160:192] = wk_eff
    consts[0:DOUT, 192] = bq_eff
    consts[0:DOUT, 193] = bk_eff
    return consts


def kernel(x, Wq, bq, Wk, bk):
    from concourse.bass_utils import run_bass_kernel_spmd

    x = np.ascontiguousarray(x, dtype=np.float32)
    consts = _host_consts(Wq, bq, Wk, bk)

    nc = _build_nc()
    in_maps = [{"x": x[i], "consts": consts} for i in range(N_CORES)]
    res = run_bass_kernel_spmd(nc, in_maps, core_ids=list(range(N_CORES)))
    global LAST_RUN
    LAST_RUN = res
    out = np.stack([r["out"] for r in res.results], axis=0)
    return out


LAST_RUN = None



# revision 12
# speedup vs baseline: 1.2100x; 1.2100x over previous
"""ChannelTimeAttention Trainium2 kernel (v2: contiguous-DMA slot layout).

Reference computation (per (b, c) pair, all independent):
    pooled = AdaptiveAvgPool(x[b, :, c]) -> [t, 8*8]      (7x7 block means)
    q = pooled @ Wq + bq ; k = pooled @ Wk + bk           [t, 32]
    att = softmax(q @ k.T / sqrt(t))                      [t, t]
    out[b, :, c] = att @ x[b, :, c].reshape(t, h*w)

Sharding: data-parallel over b — one batch element per NeuronCore (8 cores).

DMA layout (the whole point of v2): partition i = t*8 + cg (cg = c//8),
free = cl*hw + hw_idx (cl = c%8).  This makes every DMA descriptor a
CONTIGUOUS run of DRAM per partition row — measured 414 GB/s for chained
quarter reads vs ~180 GB/s for the v1 strided-pack pattern (per-engine
SDMA rate doubles with big contiguous descriptors).  x streams in as 4
column-quarter DMAs [128, 2*hw] chained on the ACT HWDGE ring; out
streams as 4 quarter DMAs on the SP HWDGE ring (separate ring so the
in-chain never queues behind a compute-waiting out descriptor).

Compute per slot j = 0..7 (channel-within-pack; channel c = cg*8 + j):
  DVE/GpSimd (alternating)  two-stage strided reduce -> pooled [128, 64]
  PE   transpose(pooled) -> pooledT [64, 128]
  PE   one fused matmul  wqk[64,64] -> q^T|k^T [64, 128] (+bias on DVE)
  PE   scoresT = k^T.T @ q^T  AND  scores = q^T.T @ k^T   [128, 128]
  DVE  +mask (block-diagonal -30), ACT exp both; scores side uses
       accum_out for the softmax denominator (no max-subtraction: folded
       weights keep |scores| << 1), DVE reciprocal -> rinv [128, 1]
  PE   out_unnorm = eT.T @ v in 7 N=448 f32r chunks; evacuation applies
       rinv as a per-partition scale (DVE tensor_scalar_mul / ACT mul)
1/49 (pool mean) and 1/sqrt(16) (score scale) are folded into Wq/bq/Wk
on the host, so no att transpose and no separate normalization pass.
"""

import numpy as np

B, T, C, H, W = 8, 16, 64, 56, 56
DS = 8
DIN = DS * DS  # 64
DOUT = 32
HW = H * W  # 3136
P = 128
NQ = 4  # input/output column-quarter DMAs
NCH = 7  # output free-dim chunks per slot
CHN = HW // NCH  # 448
N_CORES = 8
MASK_NEG = -30.0


def _build_nc():
    import concourse.bacc as bacc
    import concourse.tile as tile
    from concourse import mybir
    from concourse.masks import make_identity
    from contextlib import ExitStack

    f32 = mybir.dt.float32
    f32r = mybir.dt.float32r
    nc = bacc.Bacc(trn_type="TRN2", num_swdge_queues=2)

    # x is declared float32r end-to-end: the DMA is then a same-dtype copy
    # (HWDGE-legal) and the BIR verifier accepts it as a rounded FP32r
    # matmul operand; DVE pooling reads it bitcast back to f32.
    x_h = nc.dram_tensor("x", [T, C, H, W], f32r, kind="ExternalInput")
    # consts packed into ONE [128, 194] array (one DMA, >=512B rows):
    #   cols 0:128 mask, 128:192 wqk (rows 0:64), col 192 bias (rows 0:64)
    cn_h = nc.dram_tensor("consts", [P, 194], f32, kind="ExternalInput")
    out_h = nc.dram_tensor("out", [T, C, H, W], f32, kind="ExternalOutput")

    X = mybir.AxisListType.X
    Exp = mybir.ActivationFunctionType.Exp

    with ExitStack() as ctx:
        tc = ctx.enter_context(tile.TileContext(nc))
        singles = ctx.enter_context(tc.tile_pool(name="singles", bufs=1))
        opool = ctx.enter_context(tc.tile_pool(name="opool", bufs=3))
        small = ctx.enter_context(tc.tile_pool(name="small", bufs=3))
        psA = ctx.enter_context(tc.tile_pool(name="psA", bufs=1, space="PSUM"))
        psB = ctx.enter_context(tc.tile_pool(name="psB", bufs=3, space="PSUM"))

        consts = singles.tile([P, 194], f32)
        nc.scalar.dma_start(out=consts, in_=cn_h[:])
        mask = consts[:, 0:128]
        wq = consts[0:DIN, 128:160]
        wk = consts[0:DIN, 160:192]
        bq = consts[0:DOUT, 192:193]
        bk = consts[0:DOUT, 193:194]
        ident = singles.tile([P, P], f32)
        make_identity(nc, ident[:])

        # partition = (t, cg), free = (cl, hw): fully contiguous DRAM rows
        src = x_h[:].rearrange(
            "t (cg cl4 cl) h w -> cl4 (t cg) (cl h w)", cg=8, cl4=NQ, cl=2
        )
        dst = out_h[:].rearrange(
            "t (cg cl4 cl) h w -> cl4 (t cg) (cl h w)", cg=8, cl4=NQ, cl=2
        )

        # all 4 input quarters chained on the ACT HWDGE ring (no sem-waits
        # on input loads -> they dispatch immediately, never HOL-block ACT)
        v_tiles = []
        for q in range(NQ):
            v = singles.tile([P, 2 * HW], f32r, tag=f"v{q}", name=f"v{q}")
            nc.scalar.dma_start(out=v[:], in_=src[q])
            v_tiles.append(v)

        o_tiles = {}
        stage2 = []

        def emit_stage1(j):
            q, u = j // 2, j % 2
            v = v_tiles[q]
            eng = nc.vector  # free-axis reduce is DVE-only (gpsimd: C/XYZWC)

            # ---- adaptive avg pool (sum; /49 folded into weights) ----
            tmp = small.tile([P, H, DS], f32, tag="tmp")
            eng.reduce_sum(
                out=tmp[:],
                in_=v[:, u * HW : (u + 1) * HW]
                .bitcast(f32)
                .rearrange("p (h j vv) -> p h j vv", h=H, j=DS, vv=7),
                axis=X,
            )
            pooled = small.tile([P, DS, DS], f32, tag="pooled")
            eng.reduce_sum(
                out=pooled[:],
                in_=tmp[:].rearrange("p (i u) j -> p i j u", i=DS, u=7),
                axis=X,
            )

            # ---- pooled^T via PE so the q/k matmul contracts over d_in ----
            pT_ps = psA.tile([DIN, P], f32, tag="pT")
            nc.tensor.transpose(
                pT_ps, pooled[:].rearrange("p i j -> p (i j)"), ident
            )
            pooledT = small.tile([DIN, P], f32, tag="pooledT")
            nc.scalar.copy(pooledT, pT_ps)

            # ---- q^T, k^T [32, 128] (both at base partition 0) ----
            q_ps = psA.tile([DOUT, P], f32, tag="q")
            nc.tensor.matmul(q_ps, lhsT=wq, rhs=pooledT, start=True, stop=True)
            k_ps = psA.tile([DOUT, P], f32, tag="k")
            nc.tensor.matmul(k_ps, lhsT=wk, rhs=pooledT, start=True, stop=True)
            qT = small.tile([DOUT, P], f32, tag="qT")
            nc.vector.tensor_scalar_add(out=qT, in0=q_ps, scalar1=bq)
            kT = small.tile([DOUT, P], f32, tag="kT")
            nc.vector.tensor_scalar_add(out=kT, in0=k_ps, scalar1=bk)

            # ---- scoresT (stage-2 stationary operand) and scores (for the
            # softmax denominator); only same-cg blocks survive the mask ----
            scT_ps = psA.tile([P, P], f32, tag="scT")
            nc.tensor.matmul(scT_ps, lhsT=kT, rhs=qT, start=True, stop=True)
            sc_ps = psA.tile([P, P], f32, tag="sc")
            nc.tensor.matmul(sc_ps, lhsT=qT, rhs=kT, start=True, stop=True)
            scmT = small.tile([P, P], f32, tag="scmT")
            nc.vector.tensor_add(out=scmT, in0=scT_ps, in1=mask)
            scm = small.tile([P, P], f32, tag="scm")
            nc.vector.tensor_add(out=scm, in0=sc_ps, in1=mask)

            eT = small.tile([P, P], f32r, tag="eT")
            nc.scalar.activation(out=eT, in_=scmT, func=Exp)
            edump = small.tile([P, P], f32, tag="edump")
            ssum = small.tile([P, 1], f32, tag="ssum")
            nc.scalar.activation(out=edump, in_=scm, func=Exp, accum_out=ssum)
            rinv = small.tile([P, 1], f32, tag="rinv")
            nc.vector.reciprocal(rinv, ssum)
            stage2.append((j, eT, rinv))

        def emit_stage2(j, eT, rinv):
            q, u = j // 2, j % 2
            v = v_tiles[q]
            if u == 0:
                o_tiles[q] = opool.tile([P, 2 * HW], f32, tag="o", name="o")
                # claim the o slot with a cheap DVE op: it absorbs the WAR
                # wait on the out-DMA that previously read this slot
                nc.vector.memset(o_tiles[q][:, 0:1], 0.0)
            o = o_tiles[q]
            for ch in range(NCH):
                sl = slice(u * HW + ch * CHN, u * HW + (ch + 1) * CHN)
                ops = psB.tile([P, CHN], f32, tag="och")
                nc.tensor.matmul(
                    ops, lhsT=eT[:], rhs=v[:, sl], start=True, stop=True
                )
                # normalization folded into PSUM evacuation, split DVE/ACT
                if ch % 2 == 0:
                    nc.vector.tensor_scalar_mul(
                        out=o[:, sl], in0=ops, scalar1=rinv
                    )
                else:
                    nc.scalar.mul(o[:, sl], ops, rinv)
            if u == 1:
                # out quarters ride the SP HWDGE ring (dedicated: the
                # compute-dependent waits never block the input chain)
                nc.sync.dma_start(out=dst[q], in_=o[:])

        for j in range(2 * NQ):
            emit_stage1(j)
            if j >= 1:
                emit_stage2(*stage2[j - 1])
        emit_stage2(*stage2[2 * NQ - 1])

    nc.compile()
    return nc


def _host_consts(Wq, bq, Wk, bk):
    # fold pool-mean 1/49 into both weight mats; fold score 1/sqrt(t)=1/4
    # into the q side (weights AND bias)
    wq_eff = (Wq / (49.0 * 4.0)).astype(np.float32)
    bq_eff = (bq / 4.0).astype(np.float32)
    wk_eff = (Wk / 49.0).astype(np.float32)
    bk_eff = bk.astype(np.float32)
    # partition order (t, cg): attention pairs (i, j) belong to the same
    # channel group iff i%8 == j%8
    idx = np.arange(P)
    same_cg = np.equal.outer(idx % 8, idx % 8)
    mask = np.where(same_cg, 0.0, MASK_NEG).astype(np.float32)
    consts = np.zeros((P, 194), dtype=np.float32)
    consts[:, 0:128] = mask
    consts[0:DIN, 128:160] = wq_eff
    consts[0:DIN, 160:192] = wk_eff
    consts[0:DOUT, 192] = bq_eff
    consts[0:DOUT, 193] = bk_eff
    return consts


def kernel(x, Wq, bq, Wk, bk):
    from concourse.bass_utils import run_bass_kernel_spmd

    x = np.ascontiguousarray(x, dtype=np.float32)
    consts = _host_consts(Wq, bq, Wk, bk)

    nc = _build_nc()
    in_maps = [{"x": x[i], "consts": consts} for i in range(N_CORES)]
    res = run_bass_kernel_spmd(nc, in_maps, core_ids=list(range(N_CORES)))
    global LAST_RUN
    LAST_RUN = res
    out = np.stack([r["out"] for r in res.results], axis=0)
    return out


LAST_RUN = None


# revision 13
# speedup vs baseline: 1.3594x; 1.1235x over previous
"""ChannelTimeAttention Trainium2 kernel (v3: contiguous DMA + lean compute).

Reference computation (per (b, c) pair, all independent):
    pooled = AdaptiveAvgPool(x[b, :, c]) -> [t, 8*8]      (7x7 block means)
    q = pooled @ Wq + bq ; k = pooled @ Wk + bk           [t, 32]
    att = softmax(q @ k.T / sqrt(t))                      [t, t]
    out[b, :, c] = att @ x[b, :, c].reshape(t, h*w)

Sharding: data-parallel over b — one batch element per NeuronCore (8 cores).

DMA layout: partition i = t*8 + cg (cg = c//8), free = cl*hw + hw_idx
(cl = c%8).  Every descriptor is a contiguous >=25KB DRAM run per
partition row — measured 414 GB/s chained-quarter reads vs ~180 GB/s for
a strided-pack pattern.  x streams in as 4 column-quarter DMAs chained
on the ACT HWDGE ring (input loads carry no sem-waits, so they dispatch
immediately); out streams as 4 quarter DMAs on the SP HWDGE ring
(dedicated, so compute-dependent waits never stall the input chain).
x/v are declared float32r end-to-end: the DMA is a same-dtype copy and
the BIR verifier accepts it as a rounded FP32r matmul operand; pooling
reads v bitcast back to f32.

Compute per slot j = 0..7 (channel c = cg*8 + j):
  DVE  single fused reduce over both 7-blocks -> pooled [128, 8*8]
  PE   transpose(pooled) -> pooledT (ACT-evac to bf16) [64, 128]
  PE   q^T, k^T (bf16 weights); ACT bias-add into kA=[k;A], qB=[q;B]
       where A/B are 8 indicator rows encoding the block-diagonal mask
       as a rank-8 term: kA.T@qB = scores^T + mask, qB.T@kA = scores + mask
  ACT  exp(PSUM) -> eT (f32r, the stage-2 stationary operand); second
       exp with accum_out -> softmax denominator; DVE reciprocal.
       No max-subtraction: folded weights keep |scores| ~ 1e-5.
  PE   out_unnorm = eT.T @ v in 7 N=448 f32r chunks; PSUM evacuation
       applies 1/denominator as a per-partition scale (DVE/ACT split)
1/49 (pool mean) and 1/sqrt(16) (score scale) fold into Wq/bq/Wk host-side.
"""

import numpy as np

B, T, C, H, W = 8, 16, 64, 56, 56
DS = 8
DIN = DS * DS  # 64
DOUT = 32
HW = H * W  # 3136
P = 128
NQ = 4  # input/output column-quarter DMAs
NCH = 7  # output free-dim chunks per slot
CHN = HW // NCH  # 448
N_CORES = 8
MASK_NEG = -30.0
CW = 164  # consts width (f32 cols)


def _build_nc():
    import concourse.bacc as bacc
    import concourse.tile as tile
    from concourse import mybir
    from concourse.masks import make_identity
    from contextlib import ExitStack

    f32 = mybir.dt.float32
    f32r = mybir.dt.float32r
    bf16 = mybir.dt.bfloat16
    nc = bacc.Bacc(trn_type="TRN2", num_swdge_queues=2)

    x_h = nc.dram_tensor("x", [T, C, H, W], f32r, kind="ExternalInput")
    # consts [128, 164] f32: cols 0:16 wq-bf16(rows 0:64), 16:32 wk-bf16,
    # col 32 bq (rows 0:32), col 33 bk, cols 34:98 A-bf16 (rows 32:40),
    # cols 98:162 B-bf16 (rows 32:40)
    cn_h = nc.dram_tensor("consts", [P, CW], f32, kind="ExternalInput")
    out_h = nc.dram_tensor("out", [T, C, H, W], f32, kind="ExternalOutput")

    XY = mybir.AxisListType.XY
    Exp = mybir.ActivationFunctionType.Exp

    with ExitStack() as ctx:
        tc = ctx.enter_context(tile.TileContext(nc))
        singles = ctx.enter_context(tc.tile_pool(name="singles", bufs=1))
        opool = ctx.enter_context(tc.tile_pool(name="opool", bufs=3))
        small = ctx.enter_context(tc.tile_pool(name="small", bufs=3))
        psA = ctx.enter_context(tc.tile_pool(name="psA", bufs=1, space="PSUM"))
        psB = ctx.enter_context(tc.tile_pool(name="psB", bufs=3, space="PSUM"))

        consts = singles.tile([P, CW], f32)
        nc.scalar.dma_start(out=consts, in_=cn_h[:])
        wq_b = consts[0:DIN, 0:16].bitcast(bf16)  # [64, 32]
        wk_b = consts[0:DIN, 16:32].bitcast(bf16)
        bq = consts[0:DOUT, 32:33]
        bk = consts[0:DOUT, 33:34]
        A_b = consts[32:40, 34:98].bitcast(bf16)  # [8, 128]
        B_b = consts[32:40, 98:162].bitcast(bf16)
        ident = singles.tile([P, P], f32)
        make_identity(nc, ident[:])

        src = x_h[:].rearrange(
            "t (cg cl4 cl) h w -> cl4 (t cg) (cl h w)", cg=8, cl4=NQ, cl=2
        )
        dst = out_h[:].rearrange(
            "t (cg cl4 cl) h w -> cl4 (t cg) (cl h w)", cg=8, cl4=NQ, cl=2
        )

        v_tiles = []
        for q in range(NQ):
            v = singles.tile([P, 2 * HW], f32r, tag=f"v{q}", name=f"v{q}")
            nc.scalar.dma_start(out=v[:], in_=src[q])
            v_tiles.append(v)

        o_tiles = {}
        stage2 = []

        def emit_stage1(j):
            q, u = j // 2, j % 2
            v = v_tiles[q]

            # ---- adaptive avg pool: one fused reduce over (7h x 7w) ----
            pooled = small.tile([P, DS, DS], f32, tag="pooled")
            nc.vector.reduce_sum(
                out=pooled[:],
                in_=v[:, u * HW : (u + 1) * HW]
                .bitcast(f32)
                .rearrange(
                    "p (i u2 j vv) -> p i j u2 vv", i=DS, u2=7, j=DS, vv=7
                ),
                axis=XY,
            )

            # ---- pooled^T so the q/k matmuls contract over d_in ----
            pT_ps = psA.tile([DIN, P], f32, tag="pT")
            nc.tensor.transpose(
                pT_ps, pooled[:].rearrange("p i j -> p (i j)"), ident
            )
            pooledT = small.tile([DIN, P], bf16, tag="pooledT")
            nc.scalar.copy(pooledT, pT_ps)

            # ---- q^T, k^T [32, 128]; bias lands during ACT evacuation into
            # [40, 128] tiles whose extra 8 rows hold the mask factors ----
            q_ps = psA.tile([DOUT, P], f32, tag="q")
            nc.tensor.matmul(q_ps, lhsT=wq_b, rhs=pooledT, start=True, stop=True)
            k_ps = psA.tile([DOUT, P], f32, tag="k")
            nc.tensor.matmul(k_ps, lhsT=wk_b, rhs=pooledT, start=True, stop=True)
            qB = small.tile([40, P], bf16, tag="qB")
            kA = small.tile([40, P], bf16, tag="kA")
            nc.scalar.add(qB[0:DOUT, :], q_ps, bq)
            nc.scalar.add(kA[0:DOUT, :], k_ps, bk)
            nc.scalar.copy(qB[DOUT:40, :], B_b)
            nc.scalar.copy(kA[DOUT:40, :], A_b)

            # ---- masked scores both ways (rank-8 mask inside the matmul) --
            scT_ps = psA.tile([P, P], f32, tag="scT")
            nc.tensor.matmul(scT_ps, lhsT=kA[:], rhs=qB[:], start=True, stop=True)
            sc_ps = psA.tile([P, P], f32, tag="sc")
            nc.tensor.matmul(sc_ps, lhsT=qB[:], rhs=kA[:], start=True, stop=True)

            eT = small.tile([P, P], f32r, tag="eT")
            nc.scalar.activation(out=eT, in_=scT_ps, func=Exp)
            edump = small.tile([P, P], f32, tag="edump")
            ssum = small.tile([P, 1], f32, tag="ssum")
            nc.scalar.activation(out=edump, in_=sc_ps, func=Exp, accum_out=ssum)
            rinv = small.tile([P, 1], f32, tag="rinv")
            nc.vector.reciprocal(rinv, ssum)
            stage2.append((j, eT, rinv))

        def emit_stage2(j, eT, rinv):
            q, u = j // 2, j % 2
            v = v_tiles[q]
            if u == 0:
                o_tiles[q] = opool.tile([P, 2 * HW], f32, tag="o", name="o")
                # claim the o slot with a cheap DVE op: it absorbs the WAR
                # wait on the out-DMA that previously read this slot
                nc.vector.memset(o_tiles[q][:, 0:1], 0.0)
            o = o_tiles[q]
            for ch in range(NCH):
                sl = slice(u * HW + ch * CHN, u * HW + (ch + 1) * CHN)
                ops = psB.tile([P, CHN], f32, tag="och")
                nc.tensor.matmul(
                    ops, lhsT=eT[:], rhs=v[:, sl], start=True, stop=True
                )
                # normalization folded into PSUM evacuation, split DVE/ACT
                if ch % 3 == 0:
                    nc.vector.tensor_scalar_mul(
                        out=o[:, sl], in0=ops, scalar1=rinv
                    )
                else:
                    nc.scalar.mul(o[:, sl], ops, rinv)
            if u == 1:
                nc.sync.dma_start(out=dst[q], in_=o[:])

        for j in range(2 * NQ):
            if j >= 1:
                emit_stage2(*stage2[j - 1])
            emit_stage1(j)
        emit_stage2(*stage2[2 * NQ - 1])

    nc.compile()
    return nc


def _pack_bf16(a):
    """Pack a [r, c] f32-precision array as bf16 pairs into [r, c//2] f32."""
    import ml_dtypes

    u16 = a.astype(ml_dtypes.bfloat16).view(np.uint16)
    u32 = u16[:, 0::2].astype(np.uint32) | (
        u16[:, 1::2].astype(np.uint32) << 16
    )
    return u32.view(np.float32)


def _host_consts(Wq, bq, Wk, bk):
    # fold pool-mean 1/49 into both weight mats; fold score 1/sqrt(t)=1/4
    # into the q side (weights AND bias)
    wq_eff = (Wq / (49.0 * 4.0)).astype(np.float32)
    bq_eff = (bq / 4.0).astype(np.float32)
    wk_eff = (Wk / 49.0).astype(np.float32)
    bk_eff = bk.astype(np.float32)
    consts = np.zeros((P, CW), dtype=np.float32)
    consts[0:DIN, 0:16] = _pack_bf16(wq_eff)
    consts[0:DIN, 16:32] = _pack_bf16(wk_eff)
    consts[0:DOUT, 32] = bq_eff
    consts[0:DOUT, 33] = bk_eff
    # rank-8 mask factors: (A.T @ B)[s, t] = MASK_NEG * (s%8 != t%8)
    r = np.arange(8)[:, None]
    s = np.arange(P)[None, :]
    A = (s % 8 == r).astype(np.float32)  # [8, 128]
    Bm = MASK_NEG * (1.0 - A)  # [8, 128]
    consts[32:40, 34:98] = _pack_bf16(A)
    consts[32:40, 98:162] = _pack_bf16(Bm)
    return consts


def kernel(x, Wq, bq, Wk, bk):
    from concourse.bass_utils import run_bass_kernel_spmd

    x = np.ascontiguousarray(x, dtype=np.float32)
    consts = _host_consts(Wq, bq, Wk, bk)

    nc = _build_nc()
    in_maps = [{"x": x[i], "consts": consts} for i in range(N_CORES)]
    res = run_bass_kernel_spmd(nc, in_maps, core_ids=list(range(N_CORES)))
    global LAST_RUN
    LAST_RUN = res
    out = np.stack([r["out"] for r in res.results], axis=0)
    return out


LAST_RUN = None
